# revision 26
# baseline (speedup 1.0000x reference)
"""Trainium2 Bass kernel for batched differentiable-Markowitz layer.

Solves, for each of 2048 rows p:  min_w 0.5 w'Sigma w + p'w  s.t. w in simplex,
matching a 200-step FISTA reference (graded at rel_err < 2e-2). Structure:

  * FISTA's fixed point is independent of lr and the momentum schedule, so lr
    comes from an on-device block power-iteration bound on ||Sigma||_2.
  * 16 steps (14 bf16 matmul + 2 f32r) + a 2-iteration exact Newton polish on
    the final pre-projection iterate reach ~2e-3 relative error.
  * Lag-1 simplex threshold: w_t = relu(v_t + th_{t-1}) is emitted by ONE ACT
    op whose free-axis accumulator gives sv = sum(w_t); the Newton update
    th_t = th_{t-1} - (sv-1)/cnt runs OFF the critical path (Pool engine),
    as does the renormalization s_t = 1/sv folded into the next step's
    per-partition psum scalars scv=(1+c)s, scu=c's (renormalizing the iterate
    kills the sum-drift resonance that raw lag-1 theta excites).
  * Per step per batch tile: pw = w@A accumulates in PSUM (A = I - lr*Sigma);
    v = scv*pw - u and u' = scu*pw + lr*p on DVE; w/sv on ACT; w transposed
    on the PE; PSUM->SBUF copies split across ACT/DVE; theta/count/renorm
    row-ops on Pool.  Two batch tiles run software-skewed to overlap chains.

Sharding: data-parallel over the batch, 256 rows per core, Sigma replicated,
no collectives.
"""

import math
from contextlib import ExitStack

import numpy as np

import concourse.bass as bass  # noqa: F401
import concourse.tile as tile
from concourse import bacc, mybir
from concourse.bass_utils import run_bass_kernel_spmd

F32 = mybir.dt.float32
F32R = mybir.dt.float32r
BF16 = mybir.dt.bfloat16
OP = mybir.AluOpType
RELU = mybir.ActivationFunctionType.Relu
COPY = mybir.ActivationFunctionType.Copy

N = 256           # problem dimension
B_CORE = 256      # batch rows per core
N_CORES = 8
NB = B_CORE // 128
NK = N // 128

N_BF = 9          # bf16 matmul steps
N_MID = 2         # f32r matmul steps
N_POLISH = 0      # fp32 matmul steps (tail)
K0_NEWTON = 1     # cold-start Newton iterations (step 0)
N_FINAL = 1       # exact Newton iterations on the final v
POW_ITERS = 2
L_SAFETY = 1.25
CNT_EVERY = 4     # refresh lagged 1/cnt every k-th step


def _momentum_coeffs(n):
    # Sigma is Wishart + 0.01 I => strongly convex (kappa ~ 8); a ramped
    # constant momentum converges ~2x faster per step than FISTA's
    # 1/t^2-style schedule.
    ramp = [0.1, 0.3, 0.42]
    return ramp + [0.42] * (n + 4 - len(ramp))


def _make_identity(nc, ap, base=0):
    nc.gpsimd.memset(ap, 0.0)
    nc.gpsimd.affine_select(
        out=ap, in_=ap, compare_op=OP.not_equal, fill=1.0, base=base,
        pattern=[[-1, ap.shape[1]]], channel_multiplier=1)


def markowitz_tile_kernel(tc, out_w, in_p, in_sig, *,
                          n_bf=N_BF, n_mid=N_MID, n_polish=N_POLISH,
                          k0=K0_NEWTON, n_final=N_FINAL,
                          pow_iters=POW_ITERS, safety=L_SAFETY):
    nc = tc.nc
    ctx = ExitStack()
    n_steps = n_bf + n_mid + n_polish
    cs = _momentum_coeffs(n_steps)

    def phase_dt(t):
        if t < n_bf:
            return BF16
        if t < n_bf + n_mid:
            return F32R
        return F32

    const = ctx.enter_context(tc.tile_pool(name="const", bufs=1))
    vpool = ctx.enter_context(tc.tile_pool(name="v", bufs=3))
    upool = ctx.enter_context(tc.tile_pool(name="u", bufs=3))
    wpool = ctx.enter_context(tc.tile_pool(name="w", bufs=5))
    rpool = ctx.enter_context(tc.tile_pool(name="r", bufs=4))
    wtpool = ctx.enter_context(tc.tile_pool(name="wt", bufs=4))
    xtpool = ctx.enter_context(tc.tile_pool(name="xt", bufs=4))
    ps_w = ctx.enter_context(tc.tile_pool(name="psw", bufs=3, space="PSUM"))
    ps_t = ctx.enter_context(tc.tile_pool(name="pst", bufs=2, space="PSUM"))
    ps_m = ctx.enter_context(tc.tile_pool(name="psm", bufs=1, space="PSUM"))

    with ctx:
        # ---- persistent state ----
        S = [const.tile([128, N], F32, name=f"S{k}") for k in range(NK)]
        P = const.tile([128, NB * N], F32, name="P")   # lr*p, both tiles
        A = [const.tile([128, N], F32, name=f"A{k}") for k in range(NK)]
        A_r = [const.tile([128, N], F32R, name=f"Ar{k}") for k in range(NK)]
        A_b = [const.tile([128, N], BF16, name=f"Ab{k}") for k in range(NK)]
        IA = [const.tile([128, N], F32, name=f"IA{k}") for k in range(NK)]
        ID = const.tile([128, 128], F32, name="ID")
        ID_r = const.tile([128, 128], F32R, name="IDr")
        ID_b = const.tile([128, 128], BF16, name="IDb")
        ONES = const.tile([128, 1], F32, name="ONES")
        ONES_B = const.tile([128, 1], BF16, name="ONESB")
        # per-tile [128,1] row-op state packed as columns of [128,2] tiles so
        # the off-chain theta/renorm updates run ONCE per step pair.
        th2 = const.tile([128, 2], F32, name="th2")
        sv2 = const.tile([128, 2], F32, name="sv2")
        cv2 = const.tile([128, 2], F32, name="cv2")
        cc2 = const.tile([128, 2], F32, name="cc2")
        ic2 = const.tile([128, 2], F32, name="ic2")
        dl2 = const.tile([128, 2], F32, name="dl2")
        s2 = const.tile([128, 2], F32, name="s2")
        scv2 = const.tile([128, 2], F32, name="scv2")
        scu2 = const.tile([128, 2], F32, name="scu2")
        th = [th2[:, b:b + 1] for b in range(NB)]
        sv = [sv2[:, b:b + 1] for b in range(NB)]
        cv = [cv2[:, b:b + 1] for b in range(NB)]
        cc = [cc2[:, b:b + 1] for b in range(NB)]
        ic = [ic2[:, b:b + 1] for b in range(NB)]
        dl = [dl2[:, b:b + 1] for b in range(NB)]
        s_ = [s2[:, b:b + 1] for b in range(NB)]
        scv = [scv2[:, b:b + 1] for b in range(NB)]
        scu = [scu2[:, b:b + 1] for b in range(NB)]
        lr_vec = const.tile([128, 1], F32, name="lrv")
        nlr_vec = const.tile([128, 1], F32, name="nlrv")
        ray = const.tile([1, 32], F32, name="ray")
        ray_i = const.tile([1, 32], F32, name="rayi")
        lmax = const.tile([1, 1], F32, name="lmax")
        lsafe = const.tile([1, 1], F32, name="lsafe")
        lr_s = const.tile([1, 1], F32, name="lrs")
        nlr_s = const.tile([1, 1], F32, name="nlrs")
        w0f = const.tile([128, N], F32, name="w0f")
        csr = const.tile([1, N], F32, name="csr")
        Q0 = const.tile([128, NB * N], F32, name="Q0")
        Gv = const.tile([128, NB * N], F32, name="Gv")
        Gu = const.tile([128, NB * N], F32, name="Gu")
        C0 = const.tile([128, NB * N], F32, name="C0")
        C1 = const.tile([128, NB * N], F32, name="C1")

        # ---- load inputs ----
        for k in range(NK):
            nc.sync.dma_start(S[k][:], in_sig[128 * k:128 * (k + 1), :])
        for b in range(NB):
            nc.scalar.dma_start(P[:, N * b:N * (b + 1)],
                                in_p[128 * b:128 * (b + 1), :])

        # ---- constants ----
        _make_identity(nc, ID[:])
        nc.vector.tensor_copy(ID_r[:], ID[:])
        nc.vector.tensor_copy(ID_b[:], ID[:])
        for k in range(NK):
            _make_identity(nc, IA[k][:], base=128 * k)
        nc.gpsimd.memset(ONES[:], 1.0)
        nc.gpsimd.memset(ONES_B[:], 1.0)
        nc.gpsimd.memset(w0f[:], 1.0 / N)

        # ---- power iteration for L (bf16, transposed layout) ----
        PB = 32   # power-iteration block width (columns of Sigma)
        S_b = [const.tile([128, N], BF16, name=f"Sb{k}") for k in range(NK)]
        for k in range(NK):
            nc.vector.tensor_copy(S_b[k][:], S[k][:])
        xc = [S_b[k][:, 0:PB] for k in range(NK)]
        xp = None
        for it in range(pow_iters):
            xn = []
            for j in range(NK):
                px = ps_m.tile([128, PB], F32, tag="pps", name="pps")
                for k in range(NK):
                    nc.tensor.matmul(px[:], S_b[k][:, 128 * j:128 * (j + 1)],
                                     xc[k],
                                     start=(k == 0), stop=(k == NK - 1))
                xs = xtpool.tile([128, PB], BF16, tag="xs", name="xs")
                nc.scalar.copy(xs[:], px[:])
                xn.append(xs)
            xp, xc = xc, [t[:] for t in xn]
        pnd = ps_m.tile([1, 2 * PB], F32, tag="pps", name="pps")
        qnd = const.tile([1, 2 * PB], F32, name="qnd")
        for k in range(NK):
            prod = xtpool.tile([128, 2 * PB], BF16, tag="prodn", name="prodn")
            nc.vector.tensor_tensor(prod[:, 0:PB], xc[k], xc[k], OP.mult)
            nc.vector.tensor_tensor(prod[:, PB:2 * PB], xp[k], xc[k], OP.mult)
            nc.tensor.matmul(pnd[:], ONES_B[:], prod[:],
                             start=(k == 0), stop=(k == NK - 1))
        # column sums of Sigma (for the matmul-free step 0: w0 = 1/N)
        pcs = ps_m.tile([1, N], F32, tag="pcs", name="pcs")
        for k in range(NK):
            nc.tensor.matmul(pcs[:], ONES_B[:], S_b[k][:],
                             start=(k == 0), stop=(k == NK - 1))
        nc.scalar.copy(csr[:], pcs[:])
        for b in range(NB):
            nc.gpsimd.partition_broadcast(Q0[:, N * b:N * (b + 1)], csr[:])
        nc.gpsimd.memset(C0[:], (1.0 + cs[0]) / N)
        nc.gpsimd.memset(C1[:], cs[1] / N)
        nc.vector.scalar_tensor_tensor(Gv[:], Q0[:], (1.0 + cs[0]) / N, P[:],
                                       op0=OP.mult, op1=OP.add)
        nc.vector.scalar_tensor_tensor(Gu[:], Q0[:], cs[1] / N, P[:],
                                       op0=OP.mult, op1=OP.subtract)
        nc.scalar.copy(qnd[:], pnd[:])
        nc.vector.reciprocal(ray_i[:], qnd[:, PB:2 * PB])
        nc.vector.tensor_tensor(ray[:], qnd[:, 0:PB], ray_i[:], OP.mult)
        nc.vector.tensor_reduce(lmax[:], ray[:], axis=mybir.AxisListType.X, op=OP.max)
        nc.vector.tensor_scalar(lsafe[:], lmax[:], float(safety), None, OP.mult)
        nc.vector.reciprocal(lr_s[:], lsafe[:])
        nc.vector.tensor_scalar(nlr_s[:], lr_s[:], -1.0, None, OP.mult)
        nc.gpsimd.partition_broadcast(lr_vec[:], lr_s[:])
        nc.gpsimd.partition_broadcast(nlr_vec[:], nlr_s[:])

        # ---- A = I - lr*Sigma built per phase dtype;  P <- lr*p ----
        for k in range(NK):
            nc.vector.scalar_tensor_tensor(A_b[k][:], S[k][:], nlr_vec[:, 0:1],
                                           IA[k][:], op0=OP.mult, op1=OP.add)
        nc.vector.tensor_scalar(P[:], P[:], lr_vec[:, 0:1], None, OP.mult)
        for k in range(NK):
            nc.vector.scalar_tensor_tensor(A_r[k][:], S[k][:], nlr_vec[:, 0:1],
                                           IA[k][:], op0=OP.mult, op1=OP.add)
        if n_polish:
            for k in range(NK):
                nc.vector.scalar_tensor_tensor(A[k][:], S[k][:],
                                               nlr_vec[:, 0:1], IA[k][:],
                                               op0=OP.mult, op1=OP.add)

        # ---- initial weights: w0 = 1/N (transpose-invariant) ----
        wta = []
        for b in range(NB):
            a0 = wtpool.tile([128, N], phase_dt(0), tag=f"wta{b}", name=f"wta{b}")
            nc.vector.tensor_copy(a0[:], w0f[:])
            wta.append(a0)

        u_prev = [None] * NB
        v_last = [None] * NB

        def tile_step(b, t):
            Amm = {BF16: A_b, F32R: A_r, F32: A}[phase_dt(t)]
            # pw = w@A in PSUM
            pw = ps_w.tile([128, N], F32, tag="psW", name="psW")
            for k in range(NK):
                nc.tensor.matmul(pw[:], wta[b][:, 128 * k:128 * (k + 1)],
                                 Amm[k][:],
                                 start=(k == 0), stop=(k == NK - 1))
            # v = scv*pw - u ; u' = scu*pw + lr*p   (scv/scu carry renorm s)
            last = t == n_steps - 1
            v = vpool.tile([128, N], BF16 if t < n_bf - 1 and not last else F32,
                           tag="v", name="v")
            nc.vector.scalar_tensor_tensor(v[:], pw[:], scv[b], u_prev[b][:],
                                           op0=OP.mult, op1=OP.subtract)
            if not last:
                un = upool.tile([128, N], BF16 if t + 1 < n_bf else F32,
                                tag="u", name="u")
                nc.vector.scalar_tensor_tensor(
                    un[:], pw[:], scu[b], P[:, N * b:N * (b + 1)],
                    op0=OP.mult, op1=OP.add)
                u_prev[b] = un

            if last:
                v_last[b] = v
                return

            # w = relu(v + th) with free-axis accumulate -> sv  (ACT)
            dt_n = phase_dt(t + 1)
            w = wpool.tile([128, N], dt_n, tag="w", name="w")
            nc.scalar.activation(w[:], v[:], RELU, bias=th[b],
                                 accum_out=sv[b])

            # ---- next-step weights first: wT (PE transpose + copies) so the
            # DVE copy is not queued behind the off-chain row-ops below.
            nwa = wtpool.tile([128, N], dt_n, tag=f"wta{b}", name=f"wta{b}")
            IDmm = {BF16: ID_b, F32R: ID_r, F32: ID}[dt_n]
            pt = ps_t.tile([128, N], dt_n, tag="psT", name="psT")
            for k in range(NK):
                sl = slice(128 * k, 128 * (k + 1))
                nc.tensor.transpose(pt[:, sl], w[:, sl], IDmm[:])
                if (k + b) % 2 == 0:
                    nc.scalar.copy(nwa[:, sl], pt[:, sl])
                else:
                    nc.vector.tensor_copy(nwa[:, sl], pt[:, sl])
            wta[b] = nwa

            # ---- off-chain (merged per pair): count refresh every k steps.
            # Tile 1 runs step t-1 in pair t, so shift its test by one to
            # land both tiles' is_gt in the same pair.
            if (t + (1 if b == 1 else 0)) % CNT_EVERY == 0:
                m = rpool.tile([128, N], F32, tag="m", name="m")
                nc.vector.tensor_scalar(m[:], w[:], 0.0, None,
                                        OP.is_gt, OP.add, accum_out=cv[b])
                if b == 0:
                    nc.vector.tensor_scalar(cc2[:], cv2[:], 1.0, None, OP.max)
                    nc.vector.reciprocal(ic2[:], cc2[:])

        def cold_start():
            # Step 0 for BOTH tiles, matmul-free: w0 = 1/N uniform, so
            # pw0 = (1/N)(1 - lr*colsums(Sigma)) and
            # v0 = C0 - lr*Gv, u0 = C1 - lr*Gu with G pre-built during the
            # power iteration (the whole theta chain overlaps the A builds).
            vbs = []
            for b in range(NB):
                v = vpool.tile([128, N], BF16 if 0 < n_bf - 1 else F32,
                               tag="v", name="v")
                nc.vector.scalar_tensor_tensor(
                    v[:], Gv[:, N * b:N * (b + 1)], nlr_vec[:, 0:1],
                    C0[:, N * b:N * (b + 1)],
                    op0=OP.mult, op1=OP.add, accum_out=sv[b])
                un = upool.tile([128, N], BF16 if 1 < n_bf else F32,
                                tag="u", name="u")
                nc.vector.scalar_tensor_tensor(
                    un[:], Gu[:, N * b:N * (b + 1)], nlr_vec[:, 0:1],
                    C1[:, N * b:N * (b + 1)],
                    op0=OP.mult, op1=OP.add)
                u_prev[b] = un
                vbs.append(v[:])
            for b in range(NB):
                nc.vector.tensor_scalar(th[b], sv[b], 1.0, -1.0 / N,
                                        OP.subtract, OP.mult)
            for it in range(k0):
                for b in range(NB):
                    r = rpool.tile([128, N], F32, tag="r", name="r")
                    nc.scalar.activation(r[:], vbs[b], RELU,
                                         bias=th[b], accum_out=sv[b])
                    m = rpool.tile([128, N], F32, tag="m", name="m")
                    nc.vector.tensor_scalar(m[:], r[:], 0.0, None,
                                            OP.is_gt, OP.add, accum_out=cv[b])
                for b in range(NB):
                    nc.vector.tensor_scalar(cc[b], cv[b], 1.0, None, OP.max)
                    nc.vector.reciprocal(ic[b], cc[b])
                    nc.vector.scalar_tensor_tensor(dl[b], sv[b], 1.0, ic[b],
                                                   op0=OP.subtract, op1=OP.mult)
                    nc.vector.tensor_tensor(th[b], th[b], dl[b], OP.subtract)
            dt_n = phase_dt(1)
            IDmm = {BF16: ID_b, F32R: ID_r, F32: ID}[dt_n]
            for b in range(NB):
                # emit w with accumulate -> sv (drives scv/scu/th of step 1)
                w = wpool.tile([128, N], dt_n, tag="w", name="w")
                nc.scalar.activation(w[:], vbs[b], RELU, bias=th[b],
                                     accum_out=sv[b])
                nc.vector.scalar_tensor_tensor(dl[b], sv[b], 1.0, ic[b],
                                               op0=OP.subtract, op1=OP.mult)
                nc.vector.tensor_tensor(th[b], th[b], dl[b], OP.subtract)
                nc.vector.reciprocal(s_[b], sv[b])
                nc.vector.tensor_scalar(scv[b], s_[b], 1.0 + cs[1], None,
                                        OP.mult)
                nc.vector.tensor_scalar(scu[b], s_[b], cs[2], None, OP.mult)
                m = rpool.tile([128, N], F32, tag="m", name="m")
                nc.vector.tensor_scalar(m[:], w[:], 0.0, None,
                                        OP.is_gt, OP.add, accum_out=cv[b])
                nc.vector.tensor_scalar(cc[b], cv[b], 1.0, None, OP.max)
                nc.vector.reciprocal(ic[b], cc[b])
                nwa = wtpool.tile([128, N], dt_n, tag=f"wta{b}", name=f"wta{b}")
                pt = ps_t.tile([128, N], dt_n, tag="psT", name="psT")
                for k in range(NK):
                    sl = slice(128 * k, 128 * (k + 1))
                    nc.tensor.transpose(pt[:, sl], w[:, sl], IDmm[:])
                    if (k + b) % 2 == 0:
                        nc.scalar.copy(nwa[:, sl], pt[:, sl])
                    else:
                        nc.vector.tensor_copy(nwa[:, sl], pt[:, sl])
                wta[b] = nwa

        def polish(b):
            # Newton iteration(s) on the final v (fp32, lagged 1/cnt), then
            # emit w.  Called per tile so tile 0's polish overlaps tile 1's
            # last step.
            for it in range(n_final):
                r = rpool.tile([128, N], F32, tag="r", name="r")
                nc.scalar.activation(r[:], v_last[b][:], RELU,
                                     bias=th[b], accum_out=sv[b])
                nc.vector.scalar_tensor_tensor(dl[b], sv[b], 1.0, ic[b],
                                               op0=OP.subtract, op1=OP.mult)
                nc.vector.tensor_tensor(th[b], th[b], dl[b], OP.subtract)
            wf = wpool.tile([128, N], F32, tag="wf", name="wf")
            nc.scalar.activation(wf[:], v_last[b][:], RELU, bias=th[b])
            nc.sync.dma_start(out_w[128 * b:128 * (b + 1), :], wf[:])

        def pair_tinies(t):
            # merged theta Newton + renorm scalars for BOTH tiles (tile 0 at
            # step t, tile 1 at step t-1); one set of [128,2] row-ops.
            nc.vector.scalar_tensor_tensor(dl2[:], sv2[:], 1.0, ic2[:],
                                           op0=OP.subtract, op1=OP.mult)
            nc.vector.tensor_tensor(th2[:], th2[:], dl2[:], OP.subtract)
            nc.vector.reciprocal(s2[:], sv2[:])
            if cs[t + 1] == cs[t]:
                nc.vector.tensor_scalar(scv2[:], s2[:], 1.0 + cs[t + 1], None,
                                        OP.mult)
                nc.vector.tensor_scalar(scu2[:], s2[:], cs[t + 2], None,
                                        OP.mult)
            else:
                for b, tb in ((0, t), (1, t - 1)):
                    nc.vector.tensor_scalar(scv[b], s_[b], 1.0 + cs[tb + 1],
                                            None, OP.mult)
                    nc.vector.tensor_scalar(scu[b], s_[b], cs[tb + 2], None,
                                            OP.mult)

        # software-skewed emission: tile 1 runs one step behind tile 0.
        cold_start()
        for t in range(1, n_steps + 1):
            if t >= 2:
                tile_step(1, t - 1)
            if t < n_steps:
                tile_step(0, t)
            if t < n_steps - 1:
                pair_tinies(t)
            elif t == n_steps - 1:
                # tile 0 just ran its last step (no sv written); update only
                # tile 1's column for its final step next pair.
                nc.vector.scalar_tensor_tensor(dl[1], sv[1], 1.0, ic[1],
                                               op0=OP.subtract, op1=OP.mult)
                nc.vector.tensor_tensor(th[1], th[1], dl[1], OP.subtract)
                nc.vector.reciprocal(s_[1], sv[1])
                nc.vector.tensor_scalar(scv[1], s_[1], 1.0 + cs[t], None,
                                        OP.mult)
                polish(0)
        polish(1)


def build_nc(**kw):
    nc = bacc.Bacc("TRN2", target_bir_lowering=False, debug=False,
                   enable_asserts=False)
    p_in = nc.dram_tensor("p", [B_CORE, N], F32, kind="ExternalInput")
    s_in = nc.dram_tensor("sigma", [N, N], F32, kind="ExternalInput")
    w_out = nc.dram_tensor("w", [B_CORE, N], F32, kind="ExternalOutput")
    with tile.TileContext(nc) as tc:
        markowitz_tile_kernel(tc, w_out.ap(), p_in.ap(), s_in.ap(), **kw)
    nc.compile()
    return nc


_NC_CACHE = {}


def kernel(p_batch: np.ndarray, Sigma: np.ndarray, **kw) -> np.ndarray:
    B = p_batch.shape[0]
    rows = B // N_CORES
    assert rows == B_CORE and Sigma.shape == (N, N)
    key = tuple(sorted(kw.items()))
    if key not in _NC_CACHE:
        _NC_CACHE[key] = build_nc(**kw)
    nc = _NC_CACHE[key]
    p32 = np.ascontiguousarray(p_batch, dtype=np.float32)
    s32 = np.ascontiguousarray(Sigma, dtype=np.float32)
    in_maps = [{"p": p32[i * rows:(i + 1) * rows], "sigma": s32}
               for i in range(N_CORES)]
    res = run_bass_kernel_spmd(nc, in_maps, core_ids=list(range(N_CORES)))
    out = np.concatenate([r["w"] for r in res.results], axis=0)
    return out.astype(p_batch.dtype, copy=False)


# revision 27
# speedup vs baseline: 1.0834x; 1.0834x over previous
"""Trainium2 Bass kernel for batched differentiable-Markowitz layer.

Solves, for each of 2048 rows p:  min_w 0.5 w'Sigma w + p'w  s.t. w in simplex,
matching a 200-step FISTA reference (graded at rel_err < 2e-2). Structure:

  * FISTA's fixed point is independent of lr and the momentum schedule, so lr
    comes from an on-device block power-iteration bound on ||Sigma||_2.
  * 16 steps (14 bf16 matmul + 2 f32r) + a 2-iteration exact Newton polish on
    the final pre-projection iterate reach ~2e-3 relative error.
  * Lag-1 simplex threshold: w_t = relu(v_t + th_{t-1}) is emitted by ONE ACT
    op whose free-axis accumulator gives sv = sum(w_t); the Newton update
    th_t = th_{t-1} - (sv-1)/cnt runs OFF the critical path (Pool engine),
    as does the renormalization s_t = 1/sv folded into the next step's
    per-partition psum scalars scv=(1+c)s, scu=c's (renormalizing the iterate
    kills the sum-drift resonance that raw lag-1 theta excites).
  * Per step per batch tile: pw = w@A accumulates in PSUM (A = I - lr*Sigma);
    v = scv*pw - u and u' = scu*pw + lr*p on DVE; w/sv on ACT; w transposed
    on the PE; PSUM->SBUF copies split across ACT/DVE; theta/count/renorm
    row-ops on Pool.  Two batch tiles run software-skewed to overlap chains.

Sharding: data-parallel over the batch, 256 rows per core, Sigma replicated,
no collectives.
"""

import math
from contextlib import ExitStack

import numpy as np

import concourse.bass as bass  # noqa: F401
import concourse.tile as tile
from concourse import bacc, mybir
from concourse.bass_utils import run_bass_kernel_spmd

F32 = mybir.dt.float32
F32R = mybir.dt.float32r
BF16 = mybir.dt.bfloat16
OP = mybir.AluOpType
RELU = mybir.ActivationFunctionType.Relu
COPY = mybir.ActivationFunctionType.Copy

N = 256           # problem dimension
B_CORE = 256      # batch rows per core
N_CORES = 8
NB = B_CORE // 128
NK = N // 128

N_BF = 9          # bf16 matmul steps
N_MID = 2         # f32r matmul steps
N_POLISH = 0      # fp32 matmul steps (tail)
K0_NEWTON = 1     # cold-start Newton iterations (step 0)
N_FINAL = 1       # exact Newton iterations on the final v
POW_ITERS = 2
L_SAFETY = 1.25
CNT_EVERY = 4     # refresh lagged 1/cnt every k-th step


def _momentum_coeffs(n):
    # Sigma is Wishart + 0.01 I => strongly convex (kappa ~ 8); a ramped
    # constant momentum converges ~2x faster per step than FISTA's
    # 1/t^2-style schedule.
    ramp = [0.1, 0.3, 0.42]
    return ramp + [0.42] * (n + 4 - len(ramp))


def _make_identity(nc, ap, base=0):
    nc.gpsimd.memset(ap, 0.0)
    nc.gpsimd.affine_select(
        out=ap, in_=ap, compare_op=OP.not_equal, fill=1.0, base=base,
        pattern=[[-1, ap.shape[1]]], channel_multiplier=1)


def markowitz_tile_kernel(tc, out_w, in_p, in_sig, *,
                          n_bf=N_BF, n_mid=N_MID, n_polish=N_POLISH,
                          k0=K0_NEWTON, n_final=N_FINAL,
                          pow_iters=POW_ITERS, safety=L_SAFETY):
    nc = tc.nc
    ctx = ExitStack()
    n_steps = n_bf + n_mid + n_polish
    cs = _momentum_coeffs(n_steps)

    def phase_dt(t):
        if t < n_bf:
            return BF16
        if t < n_bf + n_mid:
            return F32R
        return F32

    const = ctx.enter_context(tc.tile_pool(name="const", bufs=1))
    vpool = ctx.enter_context(tc.tile_pool(name="v", bufs=3))
    upool = ctx.enter_context(tc.tile_pool(name="u", bufs=3))
    wpool = ctx.enter_context(tc.tile_pool(name="w", bufs=5))
    rpool = ctx.enter_context(tc.tile_pool(name="r", bufs=4))
    wtpool = ctx.enter_context(tc.tile_pool(name="wt", bufs=4))
    xtpool = ctx.enter_context(tc.tile_pool(name="xt", bufs=4))
    ps_w = ctx.enter_context(tc.tile_pool(name="psw", bufs=3, space="PSUM"))
    ps_t = ctx.enter_context(tc.tile_pool(name="pst", bufs=2, space="PSUM"))
    ps_m = ctx.enter_context(tc.tile_pool(name="psm", bufs=2, space="PSUM"))

    with ctx:
        # ---- persistent state ----
        S = [const.tile([128, N], F32, name=f"S{k}") for k in range(NK)]
        P = const.tile([128, NB * N], F32, name="P")   # lr*p, both tiles
        A = [const.tile([128, N], F32, name=f"A{k}") for k in range(NK)]
        A_r = [const.tile([128, N], F32R, name=f"Ar{k}") for k in range(NK)]
        A_b = [const.tile([128, N], BF16, name=f"Ab{k}") for k in range(NK)]
        IA = [const.tile([128, N], F32, name=f"IA{k}") for k in range(NK)]
        ID = const.tile([128, 128], F32, name="ID")
        ID_r = const.tile([128, 128], F32R, name="IDr")
        ID_b = const.tile([128, 128], BF16, name="IDb")
        ONES = const.tile([128, 1], F32, name="ONES")
        ONES_B = const.tile([128, 1], BF16, name="ONESB")
        # per-tile [128,1] row-op state packed as columns of [128,2] tiles so
        # the off-chain theta/renorm updates run ONCE per step pair.
        th2 = const.tile([128, 2], F32, name="th2")
        sv2 = const.tile([128, 2], F32, name="sv2")
        cv2 = const.tile([128, 2], F32, name="cv2")
        cc2 = const.tile([128, 2], F32, name="cc2")
        ic2 = const.tile([128, 2], F32, name="ic2")
        dl2 = const.tile([128, 2], F32, name="dl2")
        s2 = const.tile([128, 2], F32, name="s2")
        scv2 = const.tile([128, 2], F32, name="scv2")
        scu2 = const.tile([128, 2], F32, name="scu2")
        th = [th2[:, b:b + 1] for b in range(NB)]
        sv = [sv2[:, b:b + 1] for b in range(NB)]
        cv = [cv2[:, b:b + 1] for b in range(NB)]
        cc = [cc2[:, b:b + 1] for b in range(NB)]
        ic = [ic2[:, b:b + 1] for b in range(NB)]
        dl = [dl2[:, b:b + 1] for b in range(NB)]
        s_ = [s2[:, b:b + 1] for b in range(NB)]
        scv = [scv2[:, b:b + 1] for b in range(NB)]
        scu = [scu2[:, b:b + 1] for b in range(NB)]
        lr_vec = const.tile([128, 1], F32, name="lrv")
        nlr_vec = const.tile([128, 1], F32, name="nlrv")
        ray = const.tile([1, 32], F32, name="ray")
        ray_i = const.tile([1, 32], F32, name="rayi")
        lmax = const.tile([1, 1], F32, name="lmax")
        lsafe = const.tile([1, 1], F32, name="lsafe")
        lr_s = const.tile([1, 1], F32, name="lrs")
        nlr_s = const.tile([1, 1], F32, name="nlrs")
        w0f = const.tile([128, N], F32, name="w0f")

        # ---- load inputs ----
        for k in range(NK):
            nc.sync.dma_start(S[k][:], in_sig[128 * k:128 * (k + 1), :])
        for b in range(NB):
            nc.scalar.dma_start(P[:, N * b:N * (b + 1)],
                                in_p[128 * b:128 * (b + 1), :])

        # ---- constants ----
        _make_identity(nc, ID[:])
        nc.vector.tensor_copy(ID_r[:], ID[:])
        nc.vector.tensor_copy(ID_b[:], ID[:])
        for k in range(NK):
            _make_identity(nc, IA[k][:], base=128 * k)
        nc.gpsimd.memset(ONES[:], 1.0)
        nc.gpsimd.memset(ONES_B[:], 1.0)
        nc.gpsimd.memset(w0f[:], 1.0 / N)

        # ---- power iteration for L (bf16, transposed layout) ----
        PB = 32   # power-iteration block width (columns of Sigma)
        S_b = [const.tile([128, N], BF16, name=f"Sb{k}") for k in range(NK)]
        for k in range(NK):
            nc.vector.tensor_copy(S_b[k][:], S[k][:])
        xc = [S_b[k][:, 0:PB] for k in range(NK)]
        xp = None
        for it in range(pow_iters):
            xn = []
            for j in range(NK):
                px = ps_m.tile([128, PB], F32, tag="pps", name="pps")
                for k in range(NK):
                    nc.tensor.matmul(px[:], S_b[k][:, 128 * j:128 * (j + 1)],
                                     xc[k],
                                     start=(k == 0), stop=(k == NK - 1))
                xs = xtpool.tile([128, PB], BF16, tag="xs", name="xs")
                nc.scalar.copy(xs[:], px[:])
                xn.append(xs)
            xp, xc = xc, [t[:] for t in xn]
        pnd = ps_m.tile([1, 2 * PB], F32, tag="pps", name="pps")
        qnd = const.tile([1, 2 * PB], F32, name="qnd")
        for k in range(NK):
            prod = xtpool.tile([128, 2 * PB], BF16, tag="prodn", name="prodn")
            nc.vector.tensor_tensor(prod[:, 0:PB], xc[k], xc[k], OP.mult)
            nc.vector.tensor_tensor(prod[:, PB:2 * PB], xp[k], xc[k], OP.mult)
            nc.tensor.matmul(pnd[:], ONES_B[:], prod[:],
                             start=(k == 0), stop=(k == NK - 1))
        nc.scalar.copy(qnd[:], pnd[:])
        nc.vector.reciprocal(ray_i[:], qnd[:, PB:2 * PB])
        nc.vector.tensor_tensor(ray[:], qnd[:, 0:PB], ray_i[:], OP.mult)
        nc.vector.tensor_reduce(lmax[:], ray[:], axis=mybir.AxisListType.X, op=OP.max)
        nc.vector.tensor_scalar(lsafe[:], lmax[:], float(safety), None, OP.mult)
        nc.vector.reciprocal(lr_s[:], lsafe[:])
        nc.vector.tensor_scalar(nlr_s[:], lr_s[:], -1.0, None, OP.mult)
        nc.gpsimd.partition_broadcast(lr_vec[:], lr_s[:])
        nc.gpsimd.partition_broadcast(nlr_vec[:], nlr_s[:])

        # ---- A = I - lr*Sigma built per phase dtype;  P <- lr*p ----
        for k in range(NK):
            nc.vector.scalar_tensor_tensor(A_b[k][:], S[k][:], nlr_vec[:, 0:1],
                                           IA[k][:], op0=OP.mult, op1=OP.add)
        nc.vector.tensor_scalar(P[:], P[:], lr_vec[:, 0:1], None, OP.mult)
        for k in range(NK):
            nc.vector.scalar_tensor_tensor(A_r[k][:], S[k][:], nlr_vec[:, 0:1],
                                           IA[k][:], op0=OP.mult, op1=OP.add)
        if n_polish:
            for k in range(NK):
                nc.vector.scalar_tensor_tensor(A[k][:], S[k][:],
                                               nlr_vec[:, 0:1], IA[k][:],
                                               op0=OP.mult, op1=OP.add)

        # ---- initial weights: w0 = 1/N (transpose-invariant) ----
        wta = []
        for b in range(NB):
            a0 = wtpool.tile([128, N], phase_dt(0), tag=f"wta{b}", name=f"wta{b}")
            nc.vector.tensor_copy(a0[:], w0f[:])
            wta.append(a0)

        u_prev = [None] * NB
        v_last = [None] * NB

        def tile_step(b, t):
            Amm = {BF16: A_b, F32R: A_r, F32: A}[phase_dt(t)]
            # pw = w@A in PSUM
            pw = ps_w.tile([128, N], F32, tag="psW", name="psW")
            for k in range(NK):
                nc.tensor.matmul(pw[:], wta[b][:, 128 * k:128 * (k + 1)],
                                 Amm[k][:],
                                 start=(k == 0), stop=(k == NK - 1))
            # v = scv*pw - u ; u' = scu*pw + lr*p   (scv/scu carry renorm s)
            last = t == n_steps - 1
            v = vpool.tile([128, N], BF16 if t < n_bf - 1 and not last else F32,
                           tag="v", name="v")
            nc.vector.scalar_tensor_tensor(v[:], pw[:], scv[b], u_prev[b][:],
                                           op0=OP.mult, op1=OP.subtract)
            if not last:
                un = upool.tile([128, N], BF16 if t + 1 < n_bf else F32,
                                tag="u", name="u")
                nc.vector.scalar_tensor_tensor(
                    un[:], pw[:], scu[b], P[:, N * b:N * (b + 1)],
                    op0=OP.mult, op1=OP.add)
                u_prev[b] = un

            if last:
                v_last[b] = v
                return

            # w = relu(v + th) with free-axis accumulate -> sv  (ACT)
            dt_n = phase_dt(t + 1)
            w = wpool.tile([128, N], dt_n, tag="w", name="w")
            nc.scalar.activation(w[:], v[:], RELU, bias=th[b],
                                 accum_out=sv[b])

            # ---- next-step weights first: wT (PE transpose + copies) so the
            # DVE copy is not queued behind the off-chain row-ops below.
            nwa = wtpool.tile([128, N], dt_n, tag=f"wta{b}", name=f"wta{b}")
            IDmm = {BF16: ID_b, F32R: ID_r, F32: ID}[dt_n]
            pt = ps_t.tile([128, N], dt_n, tag="psT", name="psT")
            for k in range(NK):
                sl = slice(128 * k, 128 * (k + 1))
                nc.tensor.transpose(pt[:, sl], w[:, sl], IDmm[:])
                if (k + b) % 2 == 0:
                    nc.scalar.copy(nwa[:, sl], pt[:, sl])
                else:
                    nc.vector.tensor_copy(nwa[:, sl], pt[:, sl])
            wta[b] = nwa

            # ---- off-chain (merged per pair): count refresh every k steps.
            # Tile 1 runs step t-1 in pair t, so shift its test by one to
            # land both tiles' is_gt in the same pair.
            if (t + (1 if b == 1 else 0)) % CNT_EVERY == 0:
                m = rpool.tile([128, N], F32, tag="m", name="m")
                nc.vector.tensor_scalar(m[:], w[:], 0.0, None,
                                        OP.is_gt, OP.add, accum_out=cv[b])
                if b == 0:
                    nc.vector.tensor_scalar(cc2[:], cv2[:], 1.0, None, OP.max)
                    nc.vector.reciprocal(ic2[:], cc2[:])

        def cold_start():
            # Step 0 for BOTH tiles with k0 Newton iterations interleaved.
            vbs = []
            for b in range(NB):
                pw = ps_w.tile([128, N], F32, tag="psW", name="psW")
                for k in range(NK):
                    nc.tensor.matmul(pw[:], wta[b][:, 128 * k:128 * (k + 1)],
                                     A_b[k][:],
                                     start=(k == 0), stop=(k == NK - 1))
                v = vpool.tile([128, N], BF16 if 0 < n_bf - 1 else F32,
                               tag="v", name="v")
                nc.vector.scalar_tensor_tensor(
                    v[:], pw[:], 1.0 + cs[0], P[:, N * b:N * (b + 1)],
                    op0=OP.mult, op1=OP.subtract, accum_out=sv[b])
                un = upool.tile([128, N], BF16 if 1 < n_bf else F32,
                                tag="u", name="u")
                nc.vector.scalar_tensor_tensor(
                    un[:], pw[:], cs[1], P[:, N * b:N * (b + 1)],
                    op0=OP.mult, op1=OP.add)
                u_prev[b] = un
                vbs.append(v[:])
            for b in range(NB):
                nc.vector.tensor_scalar(th[b], sv[b], 1.0, -1.0 / N,
                                        OP.subtract, OP.mult)
            for it in range(k0):
                for b in range(NB):
                    r = rpool.tile([128, N], F32, tag="r", name="r")
                    nc.scalar.activation(r[:], vbs[b], RELU,
                                         bias=th[b], accum_out=sv[b])
                    m = rpool.tile([128, N], F32, tag="m", name="m")
                    nc.vector.tensor_scalar(m[:], r[:], 0.0, None,
                                            OP.is_gt, OP.add, accum_out=cv[b])
                for b in range(NB):
                    nc.vector.tensor_scalar(cc[b], cv[b], 1.0, None, OP.max)
                    nc.vector.reciprocal(ic[b], cc[b])
                    nc.vector.scalar_tensor_tensor(dl[b], sv[b], 1.0, ic[b],
                                                   op0=OP.subtract, op1=OP.mult)
                    nc.vector.tensor_tensor(th[b], th[b], dl[b], OP.subtract)
            dt_n = phase_dt(1)
            IDmm = {BF16: ID_b, F32R: ID_r, F32: ID}[dt_n]
            for b in range(NB):
                # emit w with accumulate -> sv (drives scv/scu/th of step 1)
                w = wpool.tile([128, N], dt_n, tag="w", name="w")
                nc.scalar.activation(w[:], vbs[b], RELU, bias=th[b],
                                     accum_out=sv[b])
                nc.vector.scalar_tensor_tensor(dl[b], sv[b], 1.0, ic[b],
                                               op0=OP.subtract, op1=OP.mult)
                nc.vector.tensor_tensor(th[b], th[b], dl[b], OP.subtract)
                nc.vector.reciprocal(s_[b], sv[b])
                nc.vector.tensor_scalar(scv[b], s_[b], 1.0 + cs[1], None,
                                        OP.mult)
                nc.vector.tensor_scalar(scu[b], s_[b], cs[2], None, OP.mult)
                m = rpool.tile([128, N], F32, tag="m", name="m")
                nc.vector.tensor_scalar(m[:], w[:], 0.0, None,
                                        OP.is_gt, OP.add, accum_out=cv[b])
                nc.vector.tensor_scalar(cc[b], cv[b], 1.0, None, OP.max)
                nc.vector.reciprocal(ic[b], cc[b])
                nwa = wtpool.tile([128, N], dt_n, tag=f"wta{b}", name=f"wta{b}")
                pt = ps_t.tile([128, N], dt_n, tag="psT", name="psT")
                for k in range(NK):
                    sl = slice(128 * k, 128 * (k + 1))
                    nc.tensor.transpose(pt[:, sl], w[:, sl], IDmm[:])
                    if (k + b) % 2 == 0:
                        nc.scalar.copy(nwa[:, sl], pt[:, sl])
                    else:
                        nc.vector.tensor_copy(nwa[:, sl], pt[:, sl])
                wta[b] = nwa

        def polish(b):
            # Newton iteration(s) on the final v (fp32, lagged 1/cnt), then
            # emit w.  Called per tile so tile 0's polish overlaps tile 1's
            # last step.
            for it in range(n_final):
                r = rpool.tile([128, N], F32, tag="r", name="r")
                nc.scalar.activation(r[:], v_last[b][:], RELU,
                                     bias=th[b], accum_out=sv[b])
                nc.vector.scalar_tensor_tensor(dl[b], sv[b], 1.0, ic[b],
                                               op0=OP.subtract, op1=OP.mult)
                nc.vector.tensor_tensor(th[b], th[b], dl[b], OP.subtract)
            wf = wpool.tile([128, N], F32, tag="wf", name="wf")
            nc.scalar.activation(wf[:], v_last[b][:], RELU, bias=th[b])
            nc.sync.dma_start(out_w[128 * b:128 * (b + 1), :], wf[:])

        def pair_tinies(t):
            # merged theta Newton + renorm scalars for BOTH tiles (tile 0 at
            # step t, tile 1 at step t-1); one set of [128,2] row-ops.
            nc.vector.scalar_tensor_tensor(dl2[:], sv2[:], 1.0, ic2[:],
                                           op0=OP.subtract, op1=OP.mult)
            nc.vector.tensor_tensor(th2[:], th2[:], dl2[:], OP.subtract)
            nc.vector.reciprocal(s2[:], sv2[:])
            if cs[t + 1] == cs[t]:
                nc.vector.tensor_scalar(scv2[:], s2[:], 1.0 + cs[t + 1], None,
                                        OP.mult)
                nc.vector.tensor_scalar(scu2[:], s2[:], cs[t + 2], None,
                                        OP.mult)
            else:
                for b, tb in ((0, t), (1, t - 1)):
                    nc.vector.tensor_scalar(scv[b], s_[b], 1.0 + cs[tb + 1],
                                            None, OP.mult)
                    nc.vector.tensor_scalar(scu[b], s_[b], cs[tb + 2], None,
                                            OP.mult)

        # software-skewed emission: tile 1 runs one step behind tile 0.
        cold_start()
        for t in range(1, n_steps + 1):
            if t >= 2:
                tile_step(1, t - 1)
            if t < n_steps:
                tile_step(0, t)
            if t < n_steps - 1:
                pair_tinies(t)
            elif t == n_steps - 1:
                # tile 0 just ran its last step (no sv written); update only
                # tile 1's column for its final step next pair.
                nc.vector.scalar_tensor_tensor(dl[1], sv[1], 1.0, ic[1],
                                               op0=OP.subtract, op1=OP.mult)
                nc.vector.tensor_tensor(th[1], th[1], dl[1], OP.subtract)
                nc.vector.reciprocal(s_[1], sv[1])
                nc.vector.tensor_scalar(scv[1], s_[1], 1.0 + cs[t], None,
                                        OP.mult)
                polish(0)
        polish(1)


def build_nc(**kw):
    nc = bacc.Bacc("TRN2", target_bir_lowering=False, debug=False,
                   enable_asserts=False)
    p_in = nc.dram_tensor("p", [B_CORE, N], F32, kind="ExternalInput")
    s_in = nc.dram_tensor("sigma", [N, N], F32, kind="ExternalInput")
    w_out = nc.dram_tensor("w", [B_CORE, N], F32, kind="ExternalOutput")
    with tile.TileContext(nc) as tc:
        markowitz_tile_kernel(tc, w_out.ap(), p_in.ap(), s_in.ap(), **kw)
    nc.compile()
    return nc


_NC_CACHE = {}


def kernel(p_batch: np.ndarray, Sigma: np.ndarray, **kw) -> np.ndarray:
    B = p_batch.shape[0]
    rows = B // N_CORES
    assert rows == B_CORE and Sigma.shape == (N, N)
    key = tuple(sorted(kw.items()))
    if key not in _NC_CACHE:
        _NC_CACHE[key] = build_nc(**kw)
    nc = _NC_CACHE[key]
    p32 = np.ascontiguousarray(p_batch, dtype=np.float32)
    s32 = np.ascontiguousarray(Sigma, dtype=np.float32)
    in_maps = [{"p": p32[i * rows:(i + 1) * rows], "sigma": s32}
               for i in range(N_CORES)]
    res = run_bass_kernel_spmd(nc, in_maps, core_ids=list(range(N_CORES)))
    out = np.concatenate([r["w"] for r in res.results], axis=0)
    return out.astype(p_batch.dtype, copy=False)


# revision 29
# speedup vs baseline: 1.0862x; 1.0026x over previous
"""Trainium2 Bass kernel for batched differentiable-Markowitz layer.

Solves, for each of 2048 rows p:  min_w 0.5 w'Sigma w + p'w  s.t. w in simplex,
matching a 200-step FISTA reference (graded at rel_err < 2e-2). Structure:

  * FISTA's fixed point is independent of lr and the momentum schedule, so lr
    comes from an on-device block power-iteration bound on ||Sigma||_2.
  * 16 steps (14 bf16 matmul + 2 f32r) + a 2-iteration exact Newton polish on
    the final pre-projection iterate reach ~2e-3 relative error.
  * Lag-1 simplex threshold: w_t = relu(v_t + th_{t-1}) is emitted by ONE ACT
    op whose free-axis accumulator gives sv = sum(w_t); the Newton update
    th_t = th_{t-1} - (sv-1)/cnt runs OFF the critical path (Pool engine),
    as does the renormalization s_t = 1/sv folded into the next step's
    per-partition psum scalars scv=(1+c)s, scu=c's (renormalizing the iterate
    kills the sum-drift resonance that raw lag-1 theta excites).
  * Per step per batch tile: pw = w@A accumulates in PSUM (A = I - lr*Sigma);
    v = scv*pw - u and u' = scu*pw + lr*p on DVE; w/sv on ACT; w transposed
    on the PE; PSUM->SBUF copies split across ACT/DVE; theta/count/renorm
    row-ops on Pool.  Two batch tiles run software-skewed to overlap chains.

Sharding: data-parallel over the batch, 256 rows per core, Sigma replicated,
no collectives.
"""

import math
from contextlib import ExitStack

import numpy as np

import concourse.bass as bass  # noqa: F401
import concourse.tile as tile
from concourse import bacc, mybir
from concourse.bass_utils import run_bass_kernel_spmd

F32 = mybir.dt.float32
F32R = mybir.dt.float32r
BF16 = mybir.dt.bfloat16
OP = mybir.AluOpType
RELU = mybir.ActivationFunctionType.Relu
COPY = mybir.ActivationFunctionType.Copy

N = 256           # problem dimension
B_CORE = 256      # batch rows per core
N_CORES = 8
NB = B_CORE // 128
NK = N // 128

N_BF = 9          # bf16 matmul steps
N_MID = 2         # f32r matmul steps
N_POLISH = 0      # fp32 matmul steps (tail)
K0_NEWTON = 1     # cold-start Newton iterations (step 0)
N_FINAL = 1       # exact Newton iterations on the final v
POW_ITERS = 2
L_SAFETY = 1.25
CNT_EVERY = 4     # refresh lagged 1/cnt every k-th step


def _momentum_coeffs(n):
    # Sigma is Wishart + 0.01 I => strongly convex (kappa ~ 8); a ramped
    # constant momentum converges ~2x faster per step than FISTA's
    # 1/t^2-style schedule.
    ramp = [0.1, 0.3, 0.42]
    return ramp + [0.42] * (n + 4 - len(ramp))


def _make_identity(nc, ap, base=0):
    nc.gpsimd.memset(ap, 0.0)
    nc.gpsimd.affine_select(
        out=ap, in_=ap, compare_op=OP.not_equal, fill=1.0, base=base,
        pattern=[[-1, ap.shape[1]]], channel_multiplier=1)


def markowitz_tile_kernel(tc, out_w, in_p, in_sig, *,
                          n_bf=N_BF, n_mid=N_MID, n_polish=N_POLISH,
                          k0=K0_NEWTON, n_final=N_FINAL,
                          pow_iters=POW_ITERS, safety=L_SAFETY):
    nc = tc.nc
    ctx = ExitStack()
    n_steps = n_bf + n_mid + n_polish
    cs = _momentum_coeffs(n_steps)

    def phase_dt(t):
        if t < n_bf:
            return BF16
        if t < n_bf + n_mid:
            return F32R
        return F32

    const = ctx.enter_context(tc.tile_pool(name="const", bufs=1))
    vpool = ctx.enter_context(tc.tile_pool(name="v", bufs=3))
    upool = ctx.enter_context(tc.tile_pool(name="u", bufs=3))
    wpool = ctx.enter_context(tc.tile_pool(name="w", bufs=5))
    rpool = ctx.enter_context(tc.tile_pool(name="r", bufs=4))
    wtpool = ctx.enter_context(tc.tile_pool(name="wt", bufs=4))
    xtpool = ctx.enter_context(tc.tile_pool(name="xt", bufs=4))
    ps_w = ctx.enter_context(tc.tile_pool(name="psw", bufs=3, space="PSUM"))
    ps_t = ctx.enter_context(tc.tile_pool(name="pst", bufs=2, space="PSUM"))
    ps_m = ctx.enter_context(tc.tile_pool(name="psm", bufs=2, space="PSUM"))

    with ctx:
        # ---- persistent state ----
        S = [const.tile([128, N], F32, name=f"S{k}") for k in range(NK)]
        P = const.tile([128, NB * N], F32, name="P")   # lr*p, both tiles
        A = [const.tile([128, N], F32, name=f"A{k}") for k in range(NK)]
        A_r = [const.tile([128, N], F32R, name=f"Ar{k}") for k in range(NK)]
        A_b = [const.tile([128, N], BF16, name=f"Ab{k}") for k in range(NK)]
        IA = [const.tile([128, N], F32, name=f"IA{k}") for k in range(NK)]
        ID = const.tile([128, 128], F32, name="ID")
        ID_r = const.tile([128, 128], F32R, name="IDr")
        ID_b = const.tile([128, 128], BF16, name="IDb")
        ONES = const.tile([128, 1], F32, name="ONES")
        ONES_B = const.tile([128, 1], BF16, name="ONESB")
        ONESR = const.tile([1, 128], F32, name="ONESR")
        # per-tile [128,1] row-op state packed as columns of [128,2] tiles so
        # the off-chain theta/renorm updates run ONCE per step pair.
        th2 = const.tile([128, 2], F32, name="th2")
        sv2 = const.tile([128, 2], F32, name="sv2")
        cv2 = const.tile([128, 2], F32, name="cv2")
        cc2 = const.tile([128, 2], F32, name="cc2")
        ic2 = const.tile([128, 2], F32, name="ic2")
        dl2 = const.tile([128, 2], F32, name="dl2")
        s2 = const.tile([128, 2], F32, name="s2")
        scv2 = const.tile([128, 2], F32, name="scv2")
        scu2 = const.tile([128, 2], F32, name="scu2")
        th = [th2[:, b:b + 1] for b in range(NB)]
        sv = [sv2[:, b:b + 1] for b in range(NB)]
        cv = [cv2[:, b:b + 1] for b in range(NB)]
        cc = [cc2[:, b:b + 1] for b in range(NB)]
        ic = [ic2[:, b:b + 1] for b in range(NB)]
        dl = [dl2[:, b:b + 1] for b in range(NB)]
        s_ = [s2[:, b:b + 1] for b in range(NB)]
        scv = [scv2[:, b:b + 1] for b in range(NB)]
        scu = [scu2[:, b:b + 1] for b in range(NB)]
        ray = const.tile([1, 32], F32, name="ray")
        ray_i = const.tile([1, 32], F32, name="rayi")
        lmax = const.tile([1, 1], F32, name="lmax")
        lsafe = const.tile([1, 1], F32, name="lsafe")
        lr_s = const.tile([1, 1], F32, name="lrs")
        nlr_s = const.tile([1, 1], F32, name="nlrs")
        w0f = const.tile([128, N], F32, name="w0f")

        # ---- load inputs ----
        nc.sync.dma_start(S[0][:], in_sig[0:128, :])
        nc.scalar.dma_start(S[1][:], in_sig[128:256, :])
        for b in range(NB):
            nc.gpsimd.dma_start(P[:, N * b:N * (b + 1)],
                                in_p[128 * b:128 * (b + 1), :])

        # ---- constants ----
        _make_identity(nc, ID[:])
        nc.vector.tensor_copy(ID_r[:], ID[:])
        nc.vector.tensor_copy(ID_b[:], ID[:])
        for k in range(NK):
            _make_identity(nc, IA[k][:], base=128 * k)
        nc.gpsimd.memset(ONES[:], 1.0)
        nc.gpsimd.memset(ONES_B[:], 1.0)
        nc.gpsimd.memset(ONESR[:], 1.0)
        nc.gpsimd.memset(w0f[:], 1.0 / N)

        # ---- power iteration for L (bf16, transposed layout) ----
        PB = 32   # power-iteration block width (columns of Sigma)
        S_b = [const.tile([128, N], BF16, name=f"Sb{k}") for k in range(NK)]
        for k in range(NK):
            nc.vector.tensor_copy(S_b[k][:], S[k][:])
        xc = [S_b[k][:, 0:PB] for k in range(NK)]
        xp = None
        for it in range(pow_iters):
            xn = []
            for j in range(NK):
                px = ps_m.tile([128, PB], F32, tag="pps", name="pps")
                for k in range(NK):
                    nc.tensor.matmul(px[:], S_b[k][:, 128 * j:128 * (j + 1)],
                                     xc[k],
                                     start=(k == 0), stop=(k == NK - 1))
                xs = xtpool.tile([128, PB], BF16, tag="xs", name="xs")
                nc.scalar.copy(xs[:], px[:])
                xn.append(xs)
            xp, xc = xc, [t[:] for t in xn]
        pnd = ps_m.tile([1, 2 * PB], F32, tag="pps", name="pps")
        qnd = const.tile([1, 2 * PB], F32, name="qnd")
        for k in range(NK):
            prod = xtpool.tile([128, 2 * PB], BF16, tag="prodn", name="prodn")
            nc.vector.tensor_tensor(prod[:, 0:PB], xc[k], xc[k], OP.mult)
            nc.vector.tensor_tensor(prod[:, PB:2 * PB], xp[k], xc[k], OP.mult)
            nc.tensor.matmul(pnd[:], ONES_B[:], prod[:],
                             start=(k == 0), stop=(k == NK - 1))
        nc.scalar.copy(qnd[:], pnd[:])
        nc.vector.reciprocal(ray_i[:], qnd[:, PB:2 * PB])
        nc.vector.tensor_tensor(ray[:], qnd[:, 0:PB], ray_i[:], OP.mult)
        nc.vector.tensor_reduce(lmax[:], ray[:], axis=mybir.AxisListType.X, op=OP.max)
        nc.vector.tensor_scalar(lsafe[:], lmax[:], float(safety), None, OP.mult)
        nc.vector.reciprocal(lr_s[:], lsafe[:])
        nc.vector.tensor_scalar(nlr_s[:], lr_s[:], -1.0, None, OP.mult)
        lrb = const.tile([1, 2], F32, name="lrb")
        nc.vector.tensor_copy(lrb[:, 0:1], lr_s[:])
        nc.vector.tensor_copy(lrb[:, 1:2], nlr_s[:])
        plr = ps_m.tile([128, 2], F32, tag="pps", name="plr")
        nc.tensor.matmul(plr[:], ONESR[:], lrb[:], start=True, stop=True)
        lrv2 = const.tile([128, 2], F32, name="lrv2")
        nc.scalar.copy(lrv2[:], plr[:])

        # ---- A = I - lr*Sigma built per phase dtype;  P <- lr*p ----
        for k in range(NK):
            nc.vector.scalar_tensor_tensor(A_b[k][:], S[k][:], lrv2[:, 1:2],
                                           IA[k][:], op0=OP.mult, op1=OP.add)
        nc.vector.tensor_scalar(P[:], P[:], lrv2[:, 0:1], None, OP.mult)
        for k in range(NK):
            nc.vector.scalar_tensor_tensor(A_r[k][:], S[k][:], lrv2[:, 1:2],
                                           IA[k][:], op0=OP.mult, op1=OP.add)
        if n_polish:
            for k in range(NK):
                nc.vector.scalar_tensor_tensor(A[k][:], S[k][:],
                                               lrv2[:, 1:2], IA[k][:],
                                               op0=OP.mult, op1=OP.add)

        # ---- initial weights: w0 = 1/N (transpose-invariant) ----
        wta = []
        for b in range(NB):
            a0 = wtpool.tile([128, N], phase_dt(0), tag=f"wta{b}", name=f"wta{b}")
            nc.vector.tensor_copy(a0[:], w0f[:])
            wta.append(a0)

        u_prev = [None] * NB
        v_last = [None] * NB

        def tile_step(b, t):
            Amm = {BF16: A_b, F32R: A_r, F32: A}[phase_dt(t)]
            # pw = w@A in PSUM
            pw = ps_w.tile([128, N], F32, tag="psW", name="psW")
            for k in range(NK):
                nc.tensor.matmul(pw[:], wta[b][:, 128 * k:128 * (k + 1)],
                                 Amm[k][:],
                                 start=(k == 0), stop=(k == NK - 1))
            # v = scv*pw - u ; u' = scu*pw + lr*p   (scv/scu carry renorm s)
            last = t == n_steps - 1
            v = vpool.tile([128, N], BF16 if t < n_bf - 1 and not last else F32,
                           tag="v", name="v")
            nc.vector.scalar_tensor_tensor(v[:], pw[:], scv[b], u_prev[b][:],
                                           op0=OP.mult, op1=OP.subtract)
            if not last:
                un = upool.tile([128, N], BF16 if t + 1 < n_bf else F32,
                                tag="u", name="u")
                nc.vector.scalar_tensor_tensor(
                    un[:], pw[:], scu[b], P[:, N * b:N * (b + 1)],
                    op0=OP.mult, op1=OP.add)
                u_prev[b] = un

            if last:
                v_last[b] = v
                return

            # w = relu(v + th) with free-axis accumulate -> sv  (ACT)
            dt_n = phase_dt(t + 1)
            w = wpool.tile([128, N], dt_n, tag="w", name="w")
            nc.scalar.activation(w[:], v[:], RELU, bias=th[b],
                                 accum_out=sv[b])

            # ---- next-step weights first: wT (PE transpose + copies) so the
            # DVE copy is not queued behind the off-chain row-ops below.
            nwa = wtpool.tile([128, N], dt_n, tag=f"wta{b}", name=f"wta{b}")
            IDmm = {BF16: ID_b, F32R: ID_r, F32: ID}[dt_n]
            pt = ps_t.tile([128, N], dt_n, tag="psT", name="psT")
            for k in range(NK):
                sl = slice(128 * k, 128 * (k + 1))
                nc.tensor.transpose(pt[:, sl], w[:, sl], IDmm[:])
                if (k + b) % 2 == 0:
                    nc.scalar.copy(nwa[:, sl], pt[:, sl])
                else:
                    nc.vector.tensor_copy(nwa[:, sl], pt[:, sl])
            wta[b] = nwa

            # ---- off-chain (merged per pair): count refresh every k steps.
            # Tile 1 runs step t-1 in pair t, so shift its test by one to
            # land both tiles' is_gt in the same pair.
            if (t + (1 if b == 1 else 0)) % CNT_EVERY == 0:
                m = rpool.tile([128, N], F32, tag="m", name="m")
                nc.vector.tensor_scalar(m[:], w[:], 0.0, None,
                                        OP.is_gt, OP.add, accum_out=cv[b])
                if b == 0:
                    nc.vector.tensor_scalar(cc2[:], cv2[:], 1.0, None, OP.max)
                    nc.vector.reciprocal(ic2[:], cc2[:])

        def cold_start():
            # Step 0 for BOTH tiles with k0 Newton iterations interleaved.
            vbs = []
            for b in range(NB):
                pw = ps_w.tile([128, N], F32, tag="psW", name="psW")
                for k in range(NK):
                    nc.tensor.matmul(pw[:], wta[b][:, 128 * k:128 * (k + 1)],
                                     A_b[k][:],
                                     start=(k == 0), stop=(k == NK - 1))
                v = vpool.tile([128, N], BF16 if 0 < n_bf - 1 else F32,
                               tag="v", name="v")
                nc.vector.scalar_tensor_tensor(
                    v[:], pw[:], 1.0 + cs[0], P[:, N * b:N * (b + 1)],
                    op0=OP.mult, op1=OP.subtract, accum_out=sv[b])
                un = upool.tile([128, N], BF16 if 1 < n_bf else F32,
                                tag="u", name="u")
                nc.vector.scalar_tensor_tensor(
                    un[:], pw[:], cs[1], P[:, N * b:N * (b + 1)],
                    op0=OP.mult, op1=OP.add)
                u_prev[b] = un
                vbs.append(v[:])
            for b in range(NB):
                nc.vector.tensor_scalar(th[b], sv[b], 1.0, -1.0 / N,
                                        OP.subtract, OP.mult)
            for it in range(k0):
                for b in range(NB):
                    r = rpool.tile([128, N], F32, tag="r", name="r")
                    nc.scalar.activation(r[:], vbs[b], RELU,
                                         bias=th[b], accum_out=sv[b])
                    m = rpool.tile([128, N], F32, tag="m", name="m")
                    nc.vector.tensor_scalar(m[:], r[:], 0.0, None,
                                            OP.is_gt, OP.add, accum_out=cv[b])
                for b in range(NB):
                    nc.vector.tensor_scalar(cc[b], cv[b], 1.0, None, OP.max)
                    nc.vector.reciprocal(ic[b], cc[b])
                    nc.vector.scalar_tensor_tensor(dl[b], sv[b], 1.0, ic[b],
                                                   op0=OP.subtract, op1=OP.mult)
                    nc.vector.tensor_tensor(th[b], th[b], dl[b], OP.subtract)
            dt_n = phase_dt(1)
            IDmm = {BF16: ID_b, F32R: ID_r, F32: ID}[dt_n]
            for b in range(NB):
                # emit w with accumulate -> sv (drives scv/scu/th of step 1)
                w = wpool.tile([128, N], dt_n, tag="w", name="w")
                nc.scalar.activation(w[:], vbs[b], RELU, bias=th[b],
                                     accum_out=sv[b])
                nc.vector.scalar_tensor_tensor(dl[b], sv[b], 1.0, ic[b],
                                               op0=OP.subtract, op1=OP.mult)
                nc.vector.tensor_tensor(th[b], th[b], dl[b], OP.subtract)
                nc.vector.reciprocal(s_[b], sv[b])
                nc.vector.tensor_scalar(scv[b], s_[b], 1.0 + cs[1], None,
                                        OP.mult)
                nc.vector.tensor_scalar(scu[b], s_[b], cs[2], None, OP.mult)
                m = rpool.tile([128, N], F32, tag="m", name="m")
                nc.vector.tensor_scalar(m[:], w[:], 0.0, None,
                                        OP.is_gt, OP.add, accum_out=cv[b])
                nc.vector.tensor_scalar(cc[b], cv[b], 1.0, None, OP.max)
                nc.vector.reciprocal(ic[b], cc[b])
                nwa = wtpool.tile([128, N], dt_n, tag=f"wta{b}", name=f"wta{b}")
                pt = ps_t.tile([128, N], dt_n, tag="psT", name="psT")
                for k in range(NK):
                    sl = slice(128 * k, 128 * (k + 1))
                    nc.tensor.transpose(pt[:, sl], w[:, sl], IDmm[:])
                    if (k + b) % 2 == 0:
                        nc.scalar.copy(nwa[:, sl], pt[:, sl])
                    else:
                        nc.vector.tensor_copy(nwa[:, sl], pt[:, sl])
                wta[b] = nwa

        def polish(b):
            # Newton iteration(s) on the final v (fp32, lagged 1/cnt), then
            # emit w.  Called per tile so tile 0's polish overlaps tile 1's
            # last step.
            for it in range(n_final):
                r = rpool.tile([128, N], F32, tag="r", name="r")
                nc.scalar.activation(r[:], v_last[b][:], RELU,
                                     bias=th[b], accum_out=sv[b])
                nc.vector.scalar_tensor_tensor(dl[b], sv[b], 1.0, ic[b],
                                               op0=OP.subtract, op1=OP.mult)
                nc.vector.tensor_tensor(th[b], th[b], dl[b], OP.subtract)
            wf = wpool.tile([128, N], F32, tag="wf", name="wf")
            nc.scalar.activation(wf[:], v_last[b][:], RELU, bias=th[b])
            nc.sync.dma_start(out_w[128 * b:128 * (b + 1), :], wf[:])

        def pair_tinies(t):
            # merged theta Newton + renorm scalars for BOTH tiles (tile 0 at
            # step t, tile 1 at step t-1); one set of [128,2] row-ops.
            nc.vector.scalar_tensor_tensor(dl2[:], sv2[:], 1.0, ic2[:],
                                           op0=OP.subtract, op1=OP.mult)
            nc.vector.tensor_tensor(th2[:], th2[:], dl2[:], OP.subtract)
            nc.vector.reciprocal(s2[:], sv2[:])
            if cs[t + 1] == cs[t]:
                nc.vector.tensor_scalar(scv2[:], s2[:], 1.0 + cs[t + 1], None,
                                        OP.mult)
                nc.vector.tensor_scalar(scu2[:], s2[:], cs[t + 2], None,
                                        OP.mult)
            else:
                for b, tb in ((0, t), (1, t - 1)):
                    nc.vector.tensor_scalar(scv[b], s_[b], 1.0 + cs[tb + 1],
                                            None, OP.mult)
                    nc.vector.tensor_scalar(scu[b], s_[b], cs[tb + 2], None,
                                            OP.mult)

        # software-skewed emission: tile 1 runs one step behind tile 0.
        cold_start()
        for t in range(1, n_steps + 1):
            if t >= 2:
                tile_step(1, t - 1)
            if t < n_steps:
                tile_step(0, t)
            if t < n_steps - 1:
                pair_tinies(t)
            elif t == n_steps - 1:
                # tile 0 just ran its last step (no sv written); update only
                # tile 1's column for its final step next pair.
                nc.vector.scalar_tensor_tensor(dl[1], sv[1], 1.0, ic[1],
                                               op0=OP.subtract, op1=OP.mult)
                nc.vector.tensor_tensor(th[1], th[1], dl[1], OP.subtract)
                nc.vector.reciprocal(s_[1], sv[1])
                nc.vector.tensor_scalar(scv[1], s_[1], 1.0 + cs[t], None,
                                        OP.mult)
                polish(0)
        polish(1)


def build_nc(**kw):
    nc = bacc.Bacc("TRN2", target_bir_lowering=False, debug=False,
                   enable_asserts=False)
    p_in = nc.dram_tensor("p", [B_CORE, N], F32, kind="ExternalInput")
    s_in = nc.dram_tensor("sigma", [N, N], F32, kind="ExternalInput")
    w_out = nc.dram_tensor("w", [B_CORE, N], F32, kind="ExternalOutput")
    with tile.TileContext(nc) as tc:
        markowitz_tile_kernel(tc, w_out.ap(), p_in.ap(), s_in.ap(), **kw)
    nc.compile()
    return nc


_NC_CACHE = {}


def kernel(p_batch: np.ndarray, Sigma: np.ndarray, **kw) -> np.ndarray:
    B = p_batch.shape[0]
    rows = B // N_CORES
    assert rows == B_CORE and Sigma.shape == (N, N)
    key = tuple(sorted(kw.items()))
    if key not in _NC_CACHE:
        _NC_CACHE[key] = build_nc(**kw)
    nc = _NC_CACHE[key]
    p32 = np.ascontiguousarray(p_batch, dtype=np.float32)
    s32 = np.ascontiguousarray(Sigma, dtype=np.float32)
    in_maps = [{"p": p32[i * rows:(i + 1) * rows], "sigma": s32}
               for i in range(N_CORES)]
    res = run_bass_kernel_spmd(nc, in_maps, core_ids=list(range(N_CORES)))
    out = np.concatenate([r["w"] for r in res.results], axis=0)
    return out.astype(p_batch.dtype, copy=False)


# revision 30
# speedup vs baseline: 1.0882x; 1.0019x over previous
"""Trainium2 Bass kernel for batched differentiable-Markowitz layer.

Solves, for each of 2048 rows p:  min_w 0.5 w'Sigma w + p'w  s.t. w in simplex,
matching a 200-step FISTA reference (graded at rel_err < 2e-2). Structure:

  * The fixed point is independent of lr and the momentum schedule, so lr
    comes from an on-device 32-column block power iteration (2 sweeps,
    1.25x safety) bounding ||Sigma||_2.
  * Sigma = Wishart + 0.01 I is strongly convex (kappa ~ 8), so a ramped
    CONSTANT momentum (0.1, 0.3, then 0.42) converges ~2x faster per step
    than FISTA's 1/t^2-style schedule: 11 steps (9 bf16 + 2 f32r) + a
    1-iteration exact Newton polish on the final pre-projection iterate
    reach ~3.5e-3 relative error.
  * Lag-1 simplex threshold: w_t = relu(v_t + th_{t-1}) is emitted by ONE ACT
    op whose free-axis accumulator gives sv = sum(w_t); the Newton update
    th_t = th_{t-1} - (sv-1)/cnt runs OFF the critical path, as does the
    renormalization s_t = 1/sv folded into the next step's per-partition
    psum scalars scv=(1+c)s, scu=c's (renormalizing the iterate kills the
    sum-drift resonance that raw lag-1 theta excites).  Both tiles' row-op
    state is packed in [128,2] tiles so theta/renorm updates run once per
    step pair.
  * Per step per batch tile: pw = w@A accumulates in PSUM (A = I - lr*Sigma);
    v = scv*pw - u and u' = scu*pw + lr*p on DVE; w/sv on ACT; w transposed
    on the PE; PSUM->SBUF copies split across ACT/DVE.  Two batch tiles run
    software-skewed by one step to overlap the per-step dependency chains.

Sharding: data-parallel over the batch, 256 rows per core, Sigma replicated,
no collectives.
"""

import math
from contextlib import ExitStack

import numpy as np

import concourse.bass as bass  # noqa: F401
import concourse.tile as tile
from concourse import bacc, mybir
from concourse.bass_utils import run_bass_kernel_spmd

F32 = mybir.dt.float32
F32R = mybir.dt.float32r
BF16 = mybir.dt.bfloat16
OP = mybir.AluOpType
RELU = mybir.ActivationFunctionType.Relu
COPY = mybir.ActivationFunctionType.Copy

N = 256           # problem dimension
B_CORE = 256      # batch rows per core
N_CORES = 8
NB = B_CORE // 128
NK = N // 128

N_BF = 9          # bf16 matmul steps
N_MID = 2         # f32r matmul steps
N_POLISH = 0      # fp32 matmul steps (tail)
K0_NEWTON = 1     # cold-start Newton iterations (step 0)
N_FINAL = 1       # exact Newton iterations on the final v
POW_ITERS = 2
L_SAFETY = 1.25
CNT_EVERY = 4     # refresh lagged 1/cnt every k-th step


def _momentum_coeffs(n):
    # Sigma is Wishart + 0.01 I => strongly convex (kappa ~ 8); a ramped
    # constant momentum converges ~2x faster per step than FISTA's
    # 1/t^2-style schedule.
    ramp = [0.1, 0.3, 0.42]
    return ramp + [0.42] * (n + 4 - len(ramp))


def _make_identity(nc, ap, base=0):
    nc.gpsimd.memset(ap, 0.0)
    nc.gpsimd.affine_select(
        out=ap, in_=ap, compare_op=OP.not_equal, fill=1.0, base=base,
        pattern=[[-1, ap.shape[1]]], channel_multiplier=1)


def markowitz_tile_kernel(tc, out_w, in_p, in_sig, *,
                          n_bf=N_BF, n_mid=N_MID, n_polish=N_POLISH,
                          k0=K0_NEWTON, n_final=N_FINAL,
                          pow_iters=POW_ITERS, safety=L_SAFETY):
    nc = tc.nc
    ctx = ExitStack()
    n_steps = n_bf + n_mid + n_polish
    cs = _momentum_coeffs(n_steps)

    def phase_dt(t):
        if t < n_bf:
            return BF16
        if t < n_bf + n_mid:
            return F32R
        return F32

    const = ctx.enter_context(tc.tile_pool(name="const", bufs=1))
    vpool = ctx.enter_context(tc.tile_pool(name="v", bufs=3))
    upool = ctx.enter_context(tc.tile_pool(name="u", bufs=3))
    wpool = ctx.enter_context(tc.tile_pool(name="w", bufs=5))
    rpool = ctx.enter_context(tc.tile_pool(name="r", bufs=4))
    wtpool = ctx.enter_context(tc.tile_pool(name="wt", bufs=4))
    xtpool = ctx.enter_context(tc.tile_pool(name="xt", bufs=4))
    ps_w = ctx.enter_context(tc.tile_pool(name="psw", bufs=3, space="PSUM"))
    ps_t = ctx.enter_context(tc.tile_pool(name="pst", bufs=2, space="PSUM"))
    ps_m = ctx.enter_context(tc.tile_pool(name="psm", bufs=2, space="PSUM"))

    with ctx:
        # ---- persistent state ----
        S = [const.tile([128, N], F32, name=f"S{k}") for k in range(NK)]
        P = const.tile([128, NB * N], F32, name="P")   # lr*p, both tiles
        A = [const.tile([128, N], F32, name=f"A{k}") for k in range(NK)]
        A_r = [const.tile([128, N], F32R, name=f"Ar{k}") for k in range(NK)]
        A_b = [const.tile([128, N], BF16, name=f"Ab{k}") for k in range(NK)]
        IA = [const.tile([128, N], F32, name=f"IA{k}") for k in range(NK)]
        ID = const.tile([128, 128], F32, name="ID")
        ID_r = const.tile([128, 128], F32R, name="IDr")
        ID_b = const.tile([128, 128], BF16, name="IDb")
        ONES = const.tile([128, 1], F32, name="ONES")
        ONES_B = const.tile([128, 1], BF16, name="ONESB")
        ONESR = const.tile([1, 128], F32, name="ONESR")
        # per-tile [128,1] row-op state packed as columns of [128,2] tiles so
        # the off-chain theta/renorm updates run ONCE per step pair.
        th2 = const.tile([128, 2], F32, name="th2")
        sv2 = const.tile([128, 2], F32, name="sv2")
        cv2 = const.tile([128, 2], F32, name="cv2")
        cc2 = const.tile([128, 2], F32, name="cc2")
        ic2 = const.tile([128, 2], F32, name="ic2")
        dl2 = const.tile([128, 2], F32, name="dl2")
        s2 = const.tile([128, 2], F32, name="s2")
        scv2 = const.tile([128, 2], F32, name="scv2")
        scu2 = const.tile([128, 2], F32, name="scu2")
        th = [th2[:, b:b + 1] for b in range(NB)]
        sv = [sv2[:, b:b + 1] for b in range(NB)]
        cv = [cv2[:, b:b + 1] for b in range(NB)]
        cc = [cc2[:, b:b + 1] for b in range(NB)]
        ic = [ic2[:, b:b + 1] for b in range(NB)]
        dl = [dl2[:, b:b + 1] for b in range(NB)]
        s_ = [s2[:, b:b + 1] for b in range(NB)]
        scv = [scv2[:, b:b + 1] for b in range(NB)]
        scu = [scu2[:, b:b + 1] for b in range(NB)]
        ray = const.tile([1, 32], F32, name="ray")
        ray_i = const.tile([1, 32], F32, name="rayi")
        lmax = const.tile([1, 1], F32, name="lmax")
        lsafe = const.tile([1, 1], F32, name="lsafe")
        lr_s = const.tile([1, 1], F32, name="lrs")
        nlr_s = const.tile([1, 1], F32, name="nlrs")
        w0f = const.tile([128, N], F32, name="w0f")

        # ---- load inputs ----
        nc.sync.dma_start(S[0][:], in_sig[0:128, :])
        nc.scalar.dma_start(S[1][:], in_sig[128:256, :])
        for b in range(NB):
            nc.gpsimd.dma_start(P[:, N * b:N * (b + 1)],
                                in_p[128 * b:128 * (b + 1), :])

        # ---- constants ----
        _make_identity(nc, ID[:])
        nc.vector.tensor_copy(ID_r[:], ID[:])
        nc.vector.tensor_copy(ID_b[:], ID[:])
        for k in range(NK):
            _make_identity(nc, IA[k][:], base=128 * k)
        nc.gpsimd.memset(ONES[:], 1.0)
        nc.gpsimd.memset(ONES_B[:], 1.0)
        nc.gpsimd.memset(ONESR[:], 1.0)
        nc.gpsimd.memset(w0f[:], 1.0 / N)

        # ---- power iteration for L (bf16, transposed layout) ----
        PB = 32   # power-iteration block width (columns of Sigma)
        S_b = [const.tile([128, N], BF16, name=f"Sb{k}") for k in range(NK)]
        for k in range(NK):
            nc.vector.tensor_copy(S_b[k][:], S[k][:])
        xc = [S_b[k][:, 0:PB] for k in range(NK)]
        xp = None
        for it in range(pow_iters):
            xn = []
            for j in range(NK):
                px = ps_m.tile([128, PB], F32, tag="pps", name="pps")
                for k in range(NK):
                    nc.tensor.matmul(px[:], S_b[k][:, 128 * j:128 * (j + 1)],
                                     xc[k],
                                     start=(k == 0), stop=(k == NK - 1))
                xs = xtpool.tile([128, PB], BF16, tag="xs", name="xs")
                nc.scalar.copy(xs[:], px[:])
                xn.append(xs)
            xp, xc = xc, [t[:] for t in xn]
        pnd = ps_m.tile([1, 2 * PB], F32, tag="pps", name="pps")
        qnd = const.tile([1, 2 * PB], F32, name="qnd")
        for k in range(NK):
            prod = xtpool.tile([128, 2 * PB], BF16, tag="prodn", name="prodn")
            nc.vector.tensor_tensor(prod[:, 0:PB], xc[k], xc[k], OP.mult)
            nc.vector.tensor_tensor(prod[:, PB:2 * PB], xp[k], xc[k], OP.mult)
            nc.tensor.matmul(pnd[:], ONES_B[:], prod[:],
                             start=(k == 0), stop=(k == NK - 1))
        nc.scalar.copy(qnd[:], pnd[:])
        nc.vector.reciprocal(ray_i[:], qnd[:, PB:2 * PB])
        nc.vector.tensor_tensor(ray[:], qnd[:, 0:PB], ray_i[:], OP.mult)
        nc.vector.tensor_reduce(lmax[:], ray[:], axis=mybir.AxisListType.X, op=OP.max)
        nc.vector.tensor_scalar(lsafe[:], lmax[:], float(safety), None, OP.mult)
        nc.vector.reciprocal(lr_s[:], lsafe[:])
        nc.vector.tensor_scalar(nlr_s[:], lr_s[:], -1.0, None, OP.mult)
        lrb = const.tile([1, 2], F32, name="lrb")
        nc.vector.tensor_copy(lrb[:, 0:1], lr_s[:])
        nc.vector.tensor_copy(lrb[:, 1:2], nlr_s[:])
        plr = ps_m.tile([128, 2], F32, tag="pps", name="plr")
        nc.tensor.matmul(plr[:], ONESR[:], lrb[:], start=True, stop=True)
        lrv2 = const.tile([128, 2], F32, name="lrv2")
        nc.scalar.copy(lrv2[:], plr[:])

        # ---- A = I - lr*Sigma built per phase dtype;  P <- lr*p ----
        for k in range(NK):
            nc.vector.scalar_tensor_tensor(A_b[k][:], S[k][:], lrv2[:, 1:2],
                                           IA[k][:], op0=OP.mult, op1=OP.add)
        nc.vector.tensor_scalar(P[:], P[:], lrv2[:, 0:1], None, OP.mult)
        for k in range(NK):
            nc.vector.scalar_tensor_tensor(A_r[k][:], S[k][:], lrv2[:, 1:2],
                                           IA[k][:], op0=OP.mult, op1=OP.add)
        if n_polish:
            for k in range(NK):
                nc.vector.scalar_tensor_tensor(A[k][:], S[k][:],
                                               lrv2[:, 1:2], IA[k][:],
                                               op0=OP.mult, op1=OP.add)

        # ---- initial weights: w0 = 1/N (transpose-invariant) ----
        wta = []
        for b in range(NB):
            a0 = wtpool.tile([128, N], phase_dt(0), tag=f"wta{b}", name=f"wta{b}")
            nc.vector.tensor_copy(a0[:], w0f[:])
            wta.append(a0)

        u_prev = [None] * NB
        v_last = [None] * NB

        def tile_step(b, t):
            Amm = {BF16: A_b, F32R: A_r, F32: A}[phase_dt(t)]
            # pw = w@A in PSUM
            pw = ps_w.tile([128, N], F32, tag="psW", name="psW")
            for k in range(NK):
                nc.tensor.matmul(pw[:], wta[b][:, 128 * k:128 * (k + 1)],
                                 Amm[k][:],
                                 start=(k == 0), stop=(k == NK - 1))
            # v = scv*pw - u ; u' = scu*pw + lr*p   (scv/scu carry renorm s)
            last = t == n_steps - 1
            v = vpool.tile([128, N], BF16 if t < n_bf - 1 and not last else F32,
                           tag="v", name="v")
            nc.vector.scalar_tensor_tensor(v[:], pw[:], scv[b], u_prev[b][:],
                                           op0=OP.mult, op1=OP.subtract)
            if not last:
                un = upool.tile([128, N], BF16 if t + 1 < n_bf else F32,
                                tag="u", name="u")
                nc.vector.scalar_tensor_tensor(
                    un[:], pw[:], scu[b], P[:, N * b:N * (b + 1)],
                    op0=OP.mult, op1=OP.add)
                u_prev[b] = un

            if last:
                v_last[b] = v
                return

            # w = relu(v + th) with free-axis accumulate -> sv  (ACT)
            dt_n = phase_dt(t + 1)
            w = wpool.tile([128, N], dt_n, tag="w", name="w")
            nc.scalar.activation(w[:], v[:], RELU, bias=th[b],
                                 accum_out=sv[b])

            # ---- next-step weights first: wT (PE transpose + copies) so the
            # DVE copy is not queued behind the off-chain row-ops below.
            nwa = wtpool.tile([128, N], dt_n, tag=f"wta{b}", name=f"wta{b}")
            IDmm = {BF16: ID_b, F32R: ID_r, F32: ID}[dt_n]
            pt = ps_t.tile([128, N], dt_n, tag="psT", name="psT")
            for k in range(NK):
                sl = slice(128 * k, 128 * (k + 1))
                nc.tensor.transpose(pt[:, sl], w[:, sl], IDmm[:])
                if (k + b) % 2 == 0:
                    nc.scalar.copy(nwa[:, sl], pt[:, sl])
                else:
                    nc.vector.tensor_copy(nwa[:, sl], pt[:, sl])
            wta[b] = nwa

            # ---- off-chain (merged per pair): count refresh every k steps.
            # Tile 1 runs step t-1 in pair t, so shift its test by one to
            # land both tiles' is_gt in the same pair.
            if (t + (1 if b == 1 else 0)) % CNT_EVERY == 0:
                m = rpool.tile([128, N], F32, tag="m", name="m")
                nc.vector.tensor_scalar(m[:], w[:], 0.0, None,
                                        OP.is_gt, OP.add, accum_out=cv[b])
                if b == 0:
                    nc.vector.tensor_scalar(cc2[:], cv2[:], 1.0, None, OP.max)
                    nc.vector.reciprocal(ic2[:], cc2[:])

        def cold_start():
            # Step 0 for BOTH tiles with k0 Newton iterations interleaved.
            vbs = []
            for b in range(NB):
                pw = ps_w.tile([128, N], F32, tag="psW", name="psW")
                for k in range(NK):
                    nc.tensor.matmul(pw[:], wta[b][:, 128 * k:128 * (k + 1)],
                                     A_b[k][:],
                                     start=(k == 0), stop=(k == NK - 1))
                v = vpool.tile([128, N], BF16 if 0 < n_bf - 1 else F32,
                               tag="v", name="v")
                nc.vector.scalar_tensor_tensor(
                    v[:], pw[:], 1.0 + cs[0], P[:, N * b:N * (b + 1)],
                    op0=OP.mult, op1=OP.subtract, accum_out=sv[b])
                un = upool.tile([128, N], BF16 if 1 < n_bf else F32,
                                tag="u", name="u")
                nc.vector.scalar_tensor_tensor(
                    un[:], pw[:], cs[1], P[:, N * b:N * (b + 1)],
                    op0=OP.mult, op1=OP.add)
                u_prev[b] = un
                vbs.append(v[:])
            for b in range(NB):
                nc.vector.tensor_scalar(th[b], sv[b], 1.0, -1.0 / N,
                                        OP.subtract, OP.mult)
            for it in range(k0):
                for b in range(NB):
                    r = rpool.tile([128, N], F32, tag="r", name="r")
                    nc.scalar.activation(r[:], vbs[b], RELU,
                                         bias=th[b], accum_out=sv[b])
                    m = rpool.tile([128, N], F32, tag="m", name="m")
                    nc.vector.tensor_scalar(m[:], r[:], 0.0, None,
                                            OP.is_gt, OP.add, accum_out=cv[b])
                for b in range(NB):
                    nc.vector.tensor_scalar(cc[b], cv[b], 1.0, None, OP.max)
                    nc.vector.reciprocal(ic[b], cc[b])
                    nc.vector.scalar_tensor_tensor(dl[b], sv[b], 1.0, ic[b],
                                                   op0=OP.subtract, op1=OP.mult)
                    nc.vector.tensor_tensor(th[b], th[b], dl[b], OP.subtract)
            dt_n = phase_dt(1)
            IDmm = {BF16: ID_b, F32R: ID_r, F32: ID}[dt_n]
            for b in range(NB):
                # emit w with accumulate -> sv (drives scv/scu/th of step 1)
                w = wpool.tile([128, N], dt_n, tag="w", name="w")
                nc.scalar.activation(w[:], vbs[b], RELU, bias=th[b],
                                     accum_out=sv[b])
                nc.vector.scalar_tensor_tensor(dl[b], sv[b], 1.0, ic[b],
                                               op0=OP.subtract, op1=OP.mult)
                nc.vector.tensor_tensor(th[b], th[b], dl[b], OP.subtract)
                nc.vector.reciprocal(s_[b], sv[b])
                nc.vector.tensor_scalar(scv[b], s_[b], 1.0 + cs[1], None,
                                        OP.mult)
                nc.vector.tensor_scalar(scu[b], s_[b], cs[2], None, OP.mult)
                m = rpool.tile([128, N], F32, tag="m", name="m")
                nc.vector.tensor_scalar(m[:], w[:], 0.0, None,
                                        OP.is_gt, OP.add, accum_out=cv[b])
                nc.vector.tensor_scalar(cc[b], cv[b], 1.0, None, OP.max)
                nc.vector.reciprocal(ic[b], cc[b])
                nwa = wtpool.tile([128, N], dt_n, tag=f"wta{b}", name=f"wta{b}")
                pt = ps_t.tile([128, N], dt_n, tag="psT", name="psT")
                for k in range(NK):
                    sl = slice(128 * k, 128 * (k + 1))
                    nc.tensor.transpose(pt[:, sl], w[:, sl], IDmm[:])
                    if (k + b) % 2 == 0:
                        nc.scalar.copy(nwa[:, sl], pt[:, sl])
                    else:
                        nc.vector.tensor_copy(nwa[:, sl], pt[:, sl])
                wta[b] = nwa

        def polish(b):
            # Newton iteration(s) on the final v (fp32, lagged 1/cnt), then
            # emit w.  Called per tile so tile 0's polish overlaps tile 1's
            # last step.
            for it in range(n_final):
                r = rpool.tile([128, N], F32, tag="r", name="r")
                nc.scalar.activation(r[:], v_last[b][:], RELU,
                                     bias=th[b], accum_out=sv[b])
                nc.vector.scalar_tensor_tensor(dl[b], sv[b], 1.0, ic[b],
                                               op0=OP.subtract, op1=OP.mult)
                nc.vector.tensor_tensor(th[b], th[b], dl[b], OP.subtract)
            wf = wpool.tile([128, N], F32, tag="wf", name="wf")
            nc.scalar.activation(wf[:], v_last[b][:], RELU, bias=th[b])
            nc.sync.dma_start(out_w[128 * b:128 * (b + 1), :], wf[:])

        def pair_tinies(t):
            # merged theta Newton + renorm scalars for BOTH tiles (tile 0 at
            # step t, tile 1 at step t-1); one set of [128,2] row-ops.
            nc.vector.scalar_tensor_tensor(dl2[:], sv2[:], 1.0, ic2[:],
                                           op0=OP.subtract, op1=OP.mult)
            nc.vector.tensor_tensor(th2[:], th2[:], dl2[:], OP.subtract)
            nc.vector.reciprocal(s2[:], sv2[:])
            if cs[t + 1] == cs[t]:
                nc.vector.tensor_scalar(scv2[:], s2[:], 1.0 + cs[t + 1], None,
                                        OP.mult)
                nc.vector.tensor_scalar(scu2[:], s2[:], cs[t + 2], None,
                                        OP.mult)
            else:
                for b, tb in ((0, t), (1, t - 1)):
                    nc.vector.tensor_scalar(scv[b], s_[b], 1.0 + cs[tb + 1],
                                            None, OP.mult)
                    nc.vector.tensor_scalar(scu[b], s_[b], cs[tb + 2], None,
                                            OP.mult)

        # software-skewed emission: tile 1 runs one step behind tile 0.
        cold_start()
        for t in range(1, n_steps + 1):
            if t >= 2:
                tile_step(1, t - 1)
            if t < n_steps:
                tile_step(0, t)
            if t < n_steps - 1:
                pair_tinies(t)
            elif t == n_steps - 1:
                # tile 0 just ran its last step (no sv written); update only
                # tile 1's column for its final step next pair.
                nc.vector.scalar_tensor_tensor(dl[1], sv[1], 1.0, ic[1],
                                               op0=OP.subtract, op1=OP.mult)
                nc.vector.tensor_tensor(th[1], th[1], dl[1], OP.subtract)
                nc.vector.reciprocal(s_[1], sv[1])
                nc.vector.tensor_scalar(scv[1], s_[1], 1.0 + cs[t], None,
                                        OP.mult)
                polish(0)
        polish(1)


def build_nc(**kw):
    nc = bacc.Bacc("TRN2", target_bir_lowering=False, debug=False,
                   enable_asserts=False)
    p_in = nc.dram_tensor("p", [B_CORE, N], F32, kind="ExternalInput")
    s_in = nc.dram_tensor("sigma", [N, N], F32, kind="ExternalInput")
    w_out = nc.dram_tensor("w", [B_CORE, N], F32, kind="ExternalOutput")
    with tile.TileContext(nc) as tc:
        markowitz_tile_kernel(tc, w_out.ap(), p_in.ap(), s_in.ap(), **kw)
    nc.compile()
    return nc


_NC_CACHE = {}


def kernel(p_batch: np.ndarray, Sigma: np.ndarray, **kw) -> np.ndarray:
    B = p_batch.shape[0]
    rows = B // N_CORES
    assert rows == B_CORE and Sigma.shape == (N, N)
    key = tuple(sorted(kw.items()))
    if key not in _NC_CACHE:
        _NC_CACHE[key] = build_nc(**kw)
    nc = _NC_CACHE[key]
    p32 = np.ascontiguousarray(p_batch, dtype=np.float32)
    s32 = np.ascontiguousarray(Sigma, dtype=np.float32)
    in_maps = [{"p": p32[i * rows:(i + 1) * rows], "sigma": s32}
               for i in range(N_CORES)]
    res = run_bass_kernel_spmd(nc, in_maps, core_ids=list(range(N_CORES)))
    out = np.concatenate([r["w"] for r in res.results], axis=0)
    return out.astype(p_batch.dtype, copy=False)
